# revision 1
# baseline (speedup 1.0000x reference)
"""Trainium2 Bass kernel for nn_CosineLayer (retrieval_knn).

Computes out = concat(normalize(features) @ normalize(weight).T, threshold_col).

Strategy (tensor/vocab parallel on the 434k concept axis, per sharding hint):
  - Host: L2-normalize features and weight rows (cheap one-pass prep), fold
    normalization into the weight, transpose shards to [K, N_shard] so the
    contraction dim lands on SBUF partitions, pad N to 8*54272.
  - Device (x8 SPMD): pure streaming matmul sim_shard = f_hatT.T @ w_hatT_shard
    (fp16 operands, fp32 PSUM accumulation over K=768 in 6 chunks of 128),
    DVE copy PSUM->SBUF, DMA out. HBM-roofline bound on the weight stream.
  - Host: concat shard outputs, trim padding, append threshold column.
"""

import os

import numpy as np

import concourse.mybir as mybir
import concourse.tile as tile
from concourse import bacc
from concourse.bass_utils import run_bass_kernel_spmd

N_CORES = 8
B = 256              # feature rows
K = 768              # embedding dim
KC = K // 128        # 6 k-chunks of 128 partitions
N_FULL = 434056      # concept rows
N_SHARD = 54272      # = 106*512; 8*54272 = 434176 (pad 120)
NT = int(os.environ.get("BASS_COSINE_NT", "1024"))   # n-columns per chunk
N_CHUNKS = N_SHARD // NT
OUT_BATCH = int(os.environ.get("BASS_COSINE_OUT_BATCH", "1"))  # chunks per out-DMA
EPS = 1e-8

# weight/feature compute dtype. fp16 halves HBM traffic vs fp32/fp32r and,
# with fp32 PSUM accumulation, measures 1.2e-4 scale-relative absmax vs the
# fp32 reference (fp32r measures 6.4e-5 at 1.67x the runtime; bf16 2.4e-3).
# "fp16x" additionally stores the similarity output as fp16 (host upconverts):
# another 20% traffic cut, adds <=1.1e-4 abs rounding on the largest sims.
MODE = os.environ.get("BASS_COSINE_MODE", "fp16x")
OUT_FP16 = MODE == "fp16x"

_CACHED = {}

_MODES = {
    "fp32r": (mybir.dt.float32r, np.float32),
    "fp32": (mybir.dt.float32, np.float32),
    "fp16": (mybir.dt.float16, np.float16),
    "fp16x": (mybir.dt.float16, np.float16),
    "bf16": (mybir.dt.bfloat16, None),  # np dtype resolved via ml_dtypes
}


def _np_dtype(mode):
    if mode == "bf16":
        import ml_dtypes

        return ml_dtypes.bfloat16
    return _MODES[mode][1]


def _build_bass(mode):
    """Build + compile the single-core program (same NEFF runs on all 8 cores)."""
    assert N_CHUNKS % OUT_BATCH == 0, "OUT_BATCH must divide N_CHUNKS"
    nc = bacc.Bacc("TRN2", target_bir_lowering=False, debug=False,
                   num_devices=N_CORES)
    mmdt = _MODES[mode][0]
    fT_d = nc.dram_tensor("fT", [K, B], mmdt, kind="ExternalInput").ap()
    wT_d = nc.dram_tensor("wT", [K, N_SHARD], mmdt, kind="ExternalInput").ap()
    odt = mybir.dt.float16 if OUT_FP16 else mybir.dt.float32
    out_d = nc.dram_tensor("out", [B, N_SHARD], odt, kind="ExternalOutput").ap()

    wT_r = wT_d.rearrange("(c p) n -> p c n", p=128)   # [128, KC, N_SHARD]
    fT_r = fT_d.rearrange("(c p) b -> p c b", p=128)   # [128, KC, B]

    with tile.TileContext(nc) as tc:
        with (
            tc.tile_pool(name="fpool", bufs=1) as fpool,
            tc.tile_pool(name="wpool", bufs=4) as wpool,
            tc.tile_pool(name="opool", bufs=3) as opool,
            tc.tile_pool(name="psum", bufs=4, space="PSUM") as psum,
        ):
            fsb = fpool.tile([128, KC, B], mmdt)
            nc.sync.dma_start(fsb[:], fT_r[:])

            for g in range(N_CHUNKS // OUT_BATCH):
                osb = [
                    opool.tile([128, OUT_BATCH * NT], odt,
                               name=f"osb{b}", tag=f"osb{b}")
                    for b in range(B // 128)
                ]
                for j in range(OUT_BATCH):
                    n = g * OUT_BATCH + j
                    wsb = wpool.tile([128, KC, NT], mmdt)
                    nc.sync.dma_start(wsb[:], wT_r[:, :, n * NT:(n + 1) * NT])

                    for b in range(B // 128):
                        # h innermost so both h-slices share one LDWEIGHTS
                        # per (b, c) stationary f-tile
                        pss = [
                            psum.tile([128, 512], mybir.dt.float32,
                                      name=f"ps{h}", tag=f"ps{h}")
                            for h in range(NT // 512)
                        ]
                        for c in range(KC):
                            for h in range(NT // 512):
                                nc.tensor.matmul(
                                    pss[h][:],
                                    fsb[:, c, b * 128:(b + 1) * 128],
                                    wsb[:, c, h * 512:(h + 1) * 512],
                                    start=(c == 0),
                                    stop=(c == KC - 1),
                                )
                        for h in range(NT // 512):
                            nc.vector.tensor_copy(
                                osb[b][:, j * NT + h * 512: j * NT + (h + 1) * 512],
                                pss[h][:],
                            )
                # output DMAs ride the ACT HWDGE ring so they don't
                # queue behind the next chunk's input DMA on SP
                n0 = g * OUT_BATCH * NT
                for b in range(B // 128):
                    nc.scalar.dma_start(
                        out_d[b * 128:(b + 1) * 128, n0:n0 + OUT_BATCH * NT], osb[b][:]
                    )
    nc.compile()
    return nc


def _run_spmd(nc, in_maps):
    last_exc = None
    for _ in range(3):  # device occasionally needs one recovery execute
        try:
            return run_bass_kernel_spmd(nc, in_maps, core_ids=list(range(N_CORES)))
        except Exception as e:  # noqa: BLE001
            last_exc = e
    raise last_exc


def kernel(features, weight, threshold):
    features = np.asarray(features, dtype=np.float32)
    weight = np.asarray(weight, dtype=np.float32)
    npdt = _np_dtype(MODE)

    f_norm = np.linalg.norm(features, axis=1, keepdims=True)
    f_hat = features / np.maximum(f_norm, EPS)
    fT = np.ascontiguousarray(f_hat.T).astype(npdt)          # [768, 256]

    w_norm = np.linalg.norm(weight, axis=1, keepdims=True)
    w_inv = (1.0 / np.maximum(w_norm, EPS)).astype(np.float32)

    shards = []
    for i in range(N_CORES):
        n0 = i * N_SHARD
        n1 = min(n0 + N_SHARD, N_FULL)
        s = np.zeros((K, N_SHARD), dtype=npdt)
        s[:, : n1 - n0] = (weight[n0:n1].T * w_inv[n0:n1].T).astype(npdt)
        shards.append(s)

    key = ("nc", MODE)
    if key not in _CACHED:
        _CACHED[key] = _build_bass(MODE)
    nc = _CACHED[key]

    in_maps = [{"fT": fT, "wT": shards[i]} for i in range(N_CORES)]
    res = _run_spmd(nc, in_maps)
    _CACHED["last_result"] = res

    out = np.empty((B, N_FULL + 1), dtype=np.float32)
    for i in range(N_CORES):
        n0 = i * N_SHARD
        n1 = min(n0 + N_SHARD, N_FULL)
        out[:, n0:n1] = res.results[i]["out"][:, : n1 - n0].astype(np.float32)
    out[:, N_FULL] = np.float32(threshold)
    return out



# revision 2
# speedup vs baseline: 1.1799x; 1.1799x over previous
"""Trainium2 Bass kernel for nn_CosineLayer (retrieval_knn).

Computes out = concat(normalize(features) @ normalize(weight).T, threshold_col).

Strategy (tensor/vocab parallel on the 434k concept axis, per sharding hint):
  - Host: L2-normalize features and weight rows, fold normalization into the
    weight, quantize the weight to fp8 e3m4 (x32 global scale; measured
    1.3e-2 entry rms rel err on this data -> ~1.1e-2 output rel_l2), and
    pre-swizzle each shard to [128, chunk, kc, nt] so every per-partition
    DMA line is kc*nt = 6KB contiguous.
  - Device (x8 SPMD): streaming matmul sim_shard = f_hatT.T @ w_q_shard with
    fp16 stationary features x e3m4 moving weights, fp32 PSUM accumulation
    over K=768 in 6 chunks of 128. The e3m4 weight stream halves HBM traffic
    vs fp16 (41.7MB+27.8MB out vs 111MB) and leaves the kernel at the PE
    roofline (~271us: 1 moving column/cycle at 2.4GHz).
  - Host: concat shard outputs, un-scale (/32), trim padding, append
    threshold column.
"""

import os

import numpy as np
import ml_dtypes

import concourse.mybir as mybir
import concourse.tile as tile
from concourse import bacc
from concourse.bass_utils import run_bass_kernel_spmd

N_CORES = 8
B = 256              # feature rows
K = 768              # embedding dim
KC = K // 128        # 6 k-chunks of 128 partitions
N_FULL = 434056      # concept rows
N_SHARD = 54272      # = 106*512; 8*54272 = 434176 (pad 120)
NT = int(os.environ.get("BASS_COSINE_NT", "1024"))   # n-columns per chunk
N_CHUNKS = N_SHARD // NT
OUT_BATCH = int(os.environ.get("BASS_COSINE_OUT_BATCH", "1"))  # chunks per out-DMA
EPS = 1e-8

# weight compute dtype. "e3" = fp8 e3m4 weights (x32 scale, fp16 features,
# fp16 x32-scaled output): halves weight HBM traffic vs fp16; measured
# ~1.1e-2 rel_l2 vs the fp32 reference on the seed-0 data (gate 2e-2).
# "e3s" = same but features also e3m4 (fallback if mixed-dtype matmul
# misbehaves): ~1.6e-2 rel_l2.
MODE = os.environ.get("BASS_COSINE_MODE", "e3")
W_SCALE = 32.0

_CACHED = {}


def _build_bass(mode):
    """Build + compile the single-core program (same NEFF runs on all 8 cores)."""
    assert mode in ("e3", "e3s", "fp16x")
    assert N_CHUNKS % OUT_BATCH == 0, "OUT_BATCH must divide N_CHUNKS"
    nc = bacc.Bacc("TRN2", target_bir_lowering=False, debug=False,
                   num_devices=N_CORES)
    wdt = mybir.dt.float16 if mode == "fp16x" else mybir.dt.float8e3
    fdt = mybir.dt.float8e3 if mode == "e3s" else mybir.dt.float16
    fT_d = nc.dram_tensor("fT", [K, B], fdt, kind="ExternalInput").ap()
    # pre-swizzled so chunk g is [128, KC, NT] with KC*NT contiguous per row
    wT_d = nc.dram_tensor("wT", [128, N_CHUNKS, KC, NT], wdt,
                          kind="ExternalInput").ap()
    odt = mybir.dt.float16
    out_d = nc.dram_tensor("out", [B, N_SHARD], odt, kind="ExternalOutput").ap()

    fT_r = fT_d.rearrange("(c p) b -> p c b", p=128)   # [128, KC, B]

    with tile.TileContext(nc) as tc:
        with (
            tc.tile_pool(name="fpool", bufs=1) as fpool,
            tc.tile_pool(name="wpool", bufs=4) as wpool,
            tc.tile_pool(name="opool", bufs=3) as opool,
            tc.tile_pool(name="psum", bufs=4, space="PSUM") as psum,
        ):
            fsb = fpool.tile([128, KC, B], fdt)
            nc.sync.dma_start(fsb[:], fT_r[:])

            for g in range(N_CHUNKS // OUT_BATCH):
                osb = [
                    opool.tile([128, OUT_BATCH * NT], odt,
                               name=f"osb{b}", tag=f"osb{b}")
                    for b in range(B // 128)
                ]
                for j in range(OUT_BATCH):
                    n = g * OUT_BATCH + j
                    wsb = wpool.tile([128, KC, NT], wdt)
                    nc.sync.dma_start(wsb[:], wT_d[:, n])

                    for b in range(B // 128):
                        # h innermost so both h-slices share one LDWEIGHTS
                        # per (b, c) stationary f-tile
                        pss = [
                            psum.tile([128, 512], mybir.dt.float32,
                                      name=f"ps{h}", tag=f"ps{h}")
                            for h in range(NT // 512)
                        ]
                        for c in range(KC):
                            for h in range(NT // 512):
                                nc.tensor.matmul(
                                    pss[h][:],
                                    fsb[:, c, b * 128:(b + 1) * 128],
                                    wsb[:, c, h * 512:(h + 1) * 512],
                                    start=(c == 0),
                                    stop=(c == KC - 1),
                                )
                        for h in range(NT // 512):
                            nc.vector.tensor_copy(
                                osb[b][:, j * NT + h * 512: j * NT + (h + 1) * 512],
                                pss[h][:],
                            )
                # output DMAs ride the ACT HWDGE ring so they don't
                # queue behind the next chunk's input DMA on SP
                n0 = g * OUT_BATCH * NT
                for b in range(B // 128):
                    nc.scalar.dma_start(
                        out_d[b * 128:(b + 1) * 128, n0:n0 + OUT_BATCH * NT], osb[b][:]
                    )
    nc.compile()
    return nc


def _run_spmd(nc, in_maps):
    last_exc = None
    for _ in range(3):  # device occasionally needs one recovery execute
        try:
            return run_bass_kernel_spmd(nc, in_maps, core_ids=list(range(N_CORES)))
        except Exception as e:  # noqa: BLE001
            last_exc = e
    raise last_exc


def _swizzle_shard(wq):
    """[N_SHARD, K] row-major -> [128, N_CHUNKS, KC, NT] so each chunk's
    per-partition line (KC*NT bytes) is contiguous."""
    # buf[p, g, c, t] = wq[g*NT + t, c*128 + p]
    v = wq.reshape(N_CHUNKS, NT, KC, 128)
    return np.ascontiguousarray(v.transpose(3, 0, 2, 1))


def kernel(features, weight, threshold):
    features = np.asarray(features, dtype=np.float32)
    weight = np.asarray(weight, dtype=np.float32)

    f_norm = np.linalg.norm(features, axis=1, keepdims=True)
    f_hat = features / np.maximum(f_norm, EPS)
    if MODE == "e3s":
        fT = np.ascontiguousarray(f_hat.T * W_SCALE).astype(ml_dtypes.float8_e3m4)
    else:
        fT = np.ascontiguousarray(f_hat.T).astype(np.float16)  # [768, 256]

    w_norm = np.linalg.norm(weight, axis=1, keepdims=True)
    w_inv = (W_SCALE / np.maximum(w_norm, EPS)).astype(np.float32)
    wnp = np.float16 if MODE == "fp16x" else ml_dtypes.float8_e3m4

    shards = []
    for i in range(N_CORES):
        n0 = i * N_SHARD
        n1 = min(n0 + N_SHARD, N_FULL)
        s = np.zeros((N_SHARD, K), dtype=wnp)
        s[: n1 - n0] = (weight[n0:n1] * w_inv[n0:n1]).astype(wnp)
        shards.append(_swizzle_shard(s))

    key = ("nc", MODE)
    if key not in _CACHED:
        _CACHED[key] = _build_bass(MODE)
    nc = _CACHED[key]

    in_maps = [{"fT": fT, "wT": shards[i]} for i in range(N_CORES)]
    res = _run_spmd(nc, in_maps)
    _CACHED["last_result"] = res

    inv_scale = np.float32(1.0 / W_SCALE)
    if MODE == "e3s":
        inv_scale = np.float32(1.0 / (W_SCALE * W_SCALE))
    out = np.empty((B, N_FULL + 1), dtype=np.float32)
    for i in range(N_CORES):
        n0 = i * N_SHARD
        n1 = min(n0 + N_SHARD, N_FULL)
        out[:, n0:n1] = res.results[i]["out"][:, : n1 - n0].astype(np.float32)
        out[:, n0:n1] *= inv_scale
    out[:, N_FULL] = np.float32(threshold)
    return out


# revision 7
# speedup vs baseline: 1.1910x; 1.0095x over previous
"""Trainium2 Bass kernel for nn_CosineLayer (retrieval_knn).

Computes out = concat(normalize(features) @ normalize(weight).T, threshold_col).

Strategy (tensor/vocab parallel on the 434k concept axis, per sharding hint):
  - Host: L2-normalize features and weight rows, fold normalization into the
    weight, quantize the weight to fp8 e3m4 (x32 global scale; measured
    1.3e-2 entry rms rel err on this data -> ~1.1e-2 output rel_l2), and
    pre-swizzle each shard to [128, chunk, kc, nt] so every per-partition
    DMA line is kc*nt = 6KB contiguous.
  - Device (x8 SPMD): streaming matmul sim_shard = f_hatT.T @ w_q_shard with
    fp16 stationary features x e3m4 moving weights, fp32 PSUM accumulation
    over K=768 in 6 chunks of 128. The e3m4 weight stream halves HBM traffic
    vs fp16 (41.7MB+27.8MB out vs 111MB) and leaves the kernel at the PE
    roofline (~271us: 1 moving column/cycle at 2.4GHz).
  - Host: concat shard outputs, un-scale (/32), trim padding, append
    threshold column.
"""

import os

import numpy as np
import ml_dtypes

import concourse.mybir as mybir
import concourse.tile as tile
from concourse import bacc
from concourse.bass_utils import run_bass_kernel_spmd

N_CORES = 8
B = 256              # feature rows
K = 768              # embedding dim
KC = K // 128        # 6 k-chunks of 128 partitions
N_FULL = 434056      # concept rows
N_SHARD = 54272      # = 106*512; 8*54272 = 434176 (pad 120)
NT = int(os.environ.get("BASS_COSINE_NT", "1024"))   # n-columns per chunk
N_CHUNKS = N_SHARD // NT
OUT_BATCH = int(os.environ.get("BASS_COSINE_OUT_BATCH", "1"))  # chunks per out-DMA
EPS = 1e-8

# weight compute dtype. "e3" = fp8 e3m4 weights (x32 scale, fp16 features,
# fp16 x32-scaled output): halves weight HBM traffic vs fp16; measured
# ~1.1e-2 rel_l2 vs the fp32 reference on the seed-0 data (gate 2e-2).
# "e3s" = same but features also e3m4 (fallback if mixed-dtype matmul
# misbehaves): ~1.6e-2 rel_l2.
MODE = os.environ.get("BASS_COSINE_MODE", "e3")
W_SCALE = 32.0

_CACHED = {}


def _build_bass(mode):
    """Build + compile the single-core program (same NEFF runs on all 8 cores)."""
    assert mode in ("e3", "e3s", "fp16x")
    assert N_CHUNKS % OUT_BATCH == 0, "OUT_BATCH must divide N_CHUNKS"
    nc = bacc.Bacc("TRN2", target_bir_lowering=False, debug=False,
                   num_devices=N_CORES)
    wdt = mybir.dt.float16 if mode == "fp16x" else mybir.dt.float8e3
    fdt = mybir.dt.float8e3 if mode == "e3s" else mybir.dt.float16
    fT_d = nc.dram_tensor("fT", [K, B], fdt, kind="ExternalInput").ap()
    # pre-swizzled so chunk g is [128, KC, NT] with KC*NT contiguous per row
    wT_d = nc.dram_tensor("wT", [128, N_CHUNKS, KC, NT], wdt,
                          kind="ExternalInput").ap()
    odt = mybir.dt.float16
    out_d = nc.dram_tensor("out", [B, N_SHARD], odt, kind="ExternalOutput").ap()

    fT_r = fT_d.rearrange("(c p) b -> p c b", p=128)   # [128, KC, B]

    n_warm = int(os.environ.get("BASS_COSINE_WARMUP", "8"))
    first_split = int(os.environ.get("BASS_COSINE_FIRST_SPLIT", "4"))
    assert NT % first_split == 0 and NT // first_split >= 128

    with tile.TileContext(nc) as tc:
        with (
            tc.tile_pool(name="fpool", bufs=1) as fpool,
            tc.tile_pool(name="wpool", bufs=4) as wpool,
            tc.tile_pool(name="opool", bufs=3) as opool,
            tc.tile_pool(name="psum", bufs=4, space="PSUM") as psum,
        ):
            # chunk 0 split into small pieces so the first matmul's data
            # lands ASAP; warmup matmuls on a zeroed tile ramp the PE
            # p-state out of the DMA-wait shadow.
            fnt = NT // first_split
            pieces = [(j * fnt, fnt) for j in range(first_split)]
            pieces += [(n * NT, NT) for n in range(1, N_CHUNKS)]

            wsbs = {}
            wsbs[0] = wpool.tile([128, KC, fnt], wdt, name="wsb_f0",
                                 tag="wsb_first")
            nc.sync.dma_start(wsbs[0][:], wT_d[:, 0, :, 0:fnt])

            fsb = fpool.tile([128, KC, B], fdt)
            nc.sync.dma_start(fsb[:], fT_r[:])

            if n_warm:
                wu = fpool.tile([128, 512], mybir.dt.float16, name="warm",
                                tag="warm")
                nc.any.memset(wu, 0.0)
                pwu = psum.tile([128, 512], mybir.dt.float32, name="psw",
                                tag="ps0")
                for _ in range(n_warm):
                    nc.tensor.matmul(pwu[:], wu[:, 0:128], wu[:],
                                     start=True, stop=True)

            for pi, (n0, nt) in enumerate(pieces):
                if pi not in wsbs:
                    g0 = n0 // NT
                    t0 = n0 - g0 * NT
                    tag = "wsb_first" if nt != NT else "wsbN"
                    wsbs[pi] = wpool.tile([128, KC, nt], wdt,
                                          name=f"wsb{pi}", tag=tag)
                    nc.sync.dma_start(wsbs[pi][:],
                                      wT_d[:, g0, :, t0:t0 + nt])
                if pi + 1 < len(pieces) and (pi + 1) not in wsbs:
                    n0n, ntn = pieces[pi + 1]
                    g0 = n0n // NT
                    t0 = n0n - g0 * NT
                    tag = "wsb_first" if ntn != NT else "wsbN"
                    wsbs[pi + 1] = wpool.tile([128, KC, ntn], wdt,
                                              name=f"wsb{pi + 1}", tag=tag)
                    nc.sync.dma_start(wsbs[pi + 1][:],
                                      wT_d[:, g0, :, t0:t0 + ntn])

                g = n0 // NT
                j0 = n0 - g * NT
                if j0 == 0:
                    osb = [
                        opool.tile([128, NT], odt, name=f"osb{b}", tag=f"osb{b}")
                        for b in range(B // 128)
                    ]
                wsb = wsbs.pop(pi)
                nh = max(1, nt // 512)
                hs = min(nt, 512)
                for b in range(B // 128):
                    # h innermost so both h-slices share one LDWEIGHTS
                    # per (b, c) stationary f-tile
                    pss = [
                        psum.tile([128, 512], mybir.dt.float32,
                                  name=f"ps{h}", tag=f"ps{h}")
                        for h in range(nh)
                    ]
                    for c in range(KC):
                        for h in range(nh):
                            nc.tensor.matmul(
                                pss[h][:, 0:hs],
                                fsb[:, c, b * 128:(b + 1) * 128],
                                wsb[:, c, h * hs:(h + 1) * hs],
                                start=(c == 0),
                                stop=(c == KC - 1),
                            )
                    for h in range(nh):
                        nc.vector.tensor_copy(
                            osb[b][:, j0 + h * hs: j0 + (h + 1) * hs],
                            pss[h][:, 0:hs],
                        )
                if j0 + nt == NT:
                    # output DMAs ride the ACT HWDGE ring so they don't
                    # queue behind the next chunk's input DMA on SP
                    for b in range(B // 128):
                        nc.scalar.dma_start(
                            out_d[b * 128:(b + 1) * 128, g * NT:(g + 1) * NT],
                            osb[b][:]
                        )
    nc.compile()
    return nc


def _run_spmd(nc, in_maps):
    last_exc = None
    for _ in range(3):  # device occasionally needs one recovery execute
        try:
            return run_bass_kernel_spmd(nc, in_maps, core_ids=list(range(N_CORES)))
        except Exception as e:  # noqa: BLE001
            last_exc = e
    raise last_exc


def _shards_ok(res, f_hat, weight, w_inv, inv_scale):
    """Guard against flaky device executes (observed: a run can silently
    return decorrelated garbage). Check a 128-column probe block per core
    against the host; caller reruns on failure."""
    ok = True
    for i in range(N_CORES):
        n0 = i * N_SHARD
        w_hat_blk = weight[n0:n0 + 128] * (w_inv[n0:n0 + 128] / W_SCALE)
        ref = f_hat @ w_hat_blk.T                   # [B, 128] fp32
        got = res.results[i]["out"][:, :128].astype(np.float32) * inv_scale
        err = np.abs(got - ref).max()
        if not np.isfinite(err) or err > 0.05:
            print(f"kernel self-check: core {i} probe absmax {err:.3e} "
                  f"-> rerun", flush=True)
            ok = False
    return ok


def _swizzle_shard(wq):
    """[N_SHARD, K] row-major -> [128, N_CHUNKS, KC, NT] so each chunk's
    per-partition line (KC*NT bytes) is contiguous."""
    # buf[p, g, c, t] = wq[g*NT + t, c*128 + p]
    v = wq.reshape(N_CHUNKS, NT, KC, 128)
    return np.ascontiguousarray(v.transpose(3, 0, 2, 1))


def kernel(features, weight, threshold):
    features = np.asarray(features, dtype=np.float32)
    weight = np.asarray(weight, dtype=np.float32)

    f_norm = np.linalg.norm(features, axis=1, keepdims=True)
    f_hat = features / np.maximum(f_norm, EPS)
    if MODE == "e3s":
        fT = np.ascontiguousarray(f_hat.T * W_SCALE).astype(ml_dtypes.float8_e3m4)
    else:
        fT = np.ascontiguousarray(f_hat.T).astype(np.float16)  # [768, 256]

    w_norm = np.linalg.norm(weight, axis=1, keepdims=True)
    w_inv = (W_SCALE / np.maximum(w_norm, EPS)).astype(np.float32)
    wnp = np.float16 if MODE == "fp16x" else ml_dtypes.float8_e3m4

    shards = []
    for i in range(N_CORES):
        n0 = i * N_SHARD
        n1 = min(n0 + N_SHARD, N_FULL)
        s = np.zeros((N_SHARD, K), dtype=wnp)
        s[: n1 - n0] = (weight[n0:n1] * w_inv[n0:n1]).astype(wnp)
        shards.append(_swizzle_shard(s))

    key = ("nc", MODE)
    if key not in _CACHED:
        _CACHED[key] = _build_bass(MODE)
    nc = _CACHED[key]

    inv_scale = np.float32(1.0 / W_SCALE)
    if MODE == "e3s":
        inv_scale = np.float32(1.0 / (W_SCALE * W_SCALE))

    in_maps = [{"fT": fT, "wT": shards[i]} for i in range(N_CORES)]
    res = _run_spmd(nc, in_maps)
    for _ in range(2):
        if _shards_ok(res, f_hat, weight, w_inv, inv_scale):
            break
        res = _run_spmd(nc, in_maps)
    _CACHED["last_result"] = res
    out = np.empty((B, N_FULL + 1), dtype=np.float32)
    for i in range(N_CORES):
        n0 = i * N_SHARD
        n1 = min(n0 + N_SHARD, N_FULL)
        out[:, n0:n1] = res.results[i]["out"][:, : n1 - n0].astype(np.float32)
        out[:, n0:n1] *= inv_scale
    out[:, N_FULL] = np.float32(threshold)
    return out


# revision 11
# speedup vs baseline: 1.1977x; 1.0056x over previous
"""Trainium2 Bass kernel for nn_CosineLayer (retrieval_knn).

Computes out = concat(normalize(features) @ normalize(weight).T, threshold_col).

Strategy (tensor/vocab parallel on the 434k concept axis, per sharding hint):
  - Host: L2-normalize features and weight rows, fold normalization into the
    weight, quantize the weight to fp8 e3m4 (x32 global scale; measured
    1.3e-2 entry rms rel err on this data -> ~1.1e-2 output rel_l2), and
    pre-swizzle each shard to [128, chunk, kc, nt] so every per-partition
    DMA line is kc*nt = 6KB contiguous.
  - Device (x8 SPMD): streaming matmul sim_shard = f_hatT.T @ w_q_shard with
    fp16 stationary features x e3m4 moving weights, fp32 PSUM accumulation
    over K=768 in 6 chunks of 128. The e3m4 weight stream halves HBM traffic
    vs fp16 (41.7MB+27.8MB out vs 111MB) and leaves the kernel at the PE
    roofline (~271us: 1 moving column/cycle at 2.4GHz).
  - Host: concat shard outputs, un-scale (/32), trim padding, append
    threshold column.
"""

import os

import numpy as np
import ml_dtypes

import concourse.mybir as mybir
import concourse.tile as tile
from concourse import bacc
from concourse.bass_utils import run_bass_kernel_spmd

N_CORES = 8
B = 256              # feature rows
K = 768              # embedding dim
KC = K // 128        # 6 k-chunks of 128 partitions
N_FULL = 434056      # concept rows
N_SHARD = 54272      # = 106*512; 8*54272 = 434176 (pad 120)
NT = int(os.environ.get("BASS_COSINE_NT", "1024"))   # n-columns per chunk
N_CHUNKS = N_SHARD // NT
OUT_BATCH = int(os.environ.get("BASS_COSINE_OUT_BATCH", "1"))  # chunks per out-DMA
EPS = 1e-8

# weight compute dtype. "e3" = fp8 e3m4 weights (x32 scale, fp16 features,
# fp16 x32-scaled output): halves weight HBM traffic vs fp16; measured
# ~1.1e-2 rel_l2 vs the fp32 reference on the seed-0 data (gate 2e-2).
# "e3s" = same but features also e3m4 (fallback if mixed-dtype matmul
# misbehaves): ~1.6e-2 rel_l2.
MODE = os.environ.get("BASS_COSINE_MODE", "e3")
W_SCALE = 32.0

_CACHED = {}


def _build_bass(mode):
    """Build + compile the single-core program (same NEFF runs on all 8 cores)."""
    assert mode in ("e3", "e3s", "fp16x")
    assert N_CHUNKS % OUT_BATCH == 0, "OUT_BATCH must divide N_CHUNKS"
    nc = bacc.Bacc("TRN2", target_bir_lowering=False, debug=False,
                   num_devices=N_CORES)
    wdt = mybir.dt.float16 if mode == "fp16x" else mybir.dt.float8e3
    fdt = mybir.dt.float8e3 if mode == "e3s" else mybir.dt.float16
    fT_d = nc.dram_tensor("fT", [K, B], fdt, kind="ExternalInput").ap()
    # pre-swizzled so chunk g is [128, KC, NT] with KC*NT contiguous per row
    wT_d = nc.dram_tensor("wT", [128, N_CHUNKS, KC, NT], wdt,
                          kind="ExternalInput").ap()
    odt = mybir.dt.float16
    out_d = nc.dram_tensor("out", [B, N_SHARD], odt, kind="ExternalOutput").ap()

    fT_r = fT_d.rearrange("(c p) b -> p c b", p=128)   # [128, KC, B]

    n_warm = int(os.environ.get("BASS_COSINE_WARMUP", "14"))
    first_split = int(os.environ.get("BASS_COSINE_FIRST_SPLIT", "4"))
    assert NT % first_split == 0 and NT // first_split >= 128

    with tile.TileContext(nc) as tc:
        with (
            tc.tile_pool(name="fpool", bufs=1) as fpool,
            tc.tile_pool(name="wpool", bufs=4) as wpool,
            tc.tile_pool(name="opool", bufs=3) as opool,
            tc.tile_pool(name="psum", bufs=4, space="PSUM") as psum,
        ):
            # chunk 0 split into small pieces so the first matmul's data
            # lands ASAP; warmup matmuls on a zeroed tile ramp the PE
            # p-state out of the DMA-wait shadow.
            fnt = NT // first_split
            pieces = [(j * fnt, fnt) for j in range(first_split)]
            pieces += [(n * NT, NT) for n in range(1, N_CHUNKS - 1)]
            # last chunk split too: its output drains per piece, shrinking
            # the end-of-kernel CAST+DMA tail
            last0 = (N_CHUNKS - 1) * NT
            pieces += [(last0 + j * fnt, fnt) for j in range(first_split)]

            wsbs = {}
            wsbs[0] = wpool.tile([128, KC, fnt], wdt, name="wsb_f0",
                                 tag="wsb_first")
            nc.sync.dma_start(wsbs[0][:], wT_d[:, 0, :, 0:fnt])

            fsb = fpool.tile([128, KC, B], fdt)
            nc.sync.dma_start(fsb[:], fT_r[:])

            if n_warm:
                wu = fpool.tile([128, 512], mybir.dt.float16, name="warm",
                                tag="warm")
                nc.any.memset(wu, 0.0)
                pwu = psum.tile([128, 512], mybir.dt.float32, name="psw",
                                tag="ps0")
                for _ in range(n_warm):
                    nc.tensor.matmul(pwu[:], wu[:, 0:128], wu[:],
                                     start=True, stop=True)

            for pi, (n0, nt) in enumerate(pieces):
                if pi not in wsbs:
                    g0 = n0 // NT
                    t0 = n0 - g0 * NT
                    tag = "wsb_first" if nt != NT else "wsbN"
                    wsbs[pi] = wpool.tile([128, KC, nt], wdt,
                                          name=f"wsb{pi}", tag=tag)
                    nc.sync.dma_start(wsbs[pi][:],
                                      wT_d[:, g0, :, t0:t0 + nt])
                if pi + 1 < len(pieces) and (pi + 1) not in wsbs:
                    n0n, ntn = pieces[pi + 1]
                    g0 = n0n // NT
                    t0 = n0n - g0 * NT
                    tag = "wsb_first" if ntn != NT else "wsbN"
                    wsbs[pi + 1] = wpool.tile([128, KC, ntn], wdt,
                                              name=f"wsb{pi + 1}", tag=tag)
                    nc.sync.dma_start(wsbs[pi + 1][:],
                                      wT_d[:, g0, :, t0:t0 + ntn])

                g = n0 // NT
                j0 = n0 - g * NT
                if j0 == 0:
                    osb = [
                        opool.tile([128, NT], odt, name=f"osb{b}", tag=f"osb{b}")
                        for b in range(B // 128)
                    ]
                wsb = wsbs.pop(pi)
                nh = max(1, nt // 512)
                hs = min(nt, 512)
                for b in range(B // 128):
                    # h innermost so both h-slices share one LDWEIGHTS
                    # per (b, c) stationary f-tile
                    pss = [
                        psum.tile([128, 512], mybir.dt.float32,
                                  name=f"ps{h}", tag=f"ps{h}")
                        for h in range(nh)
                    ]
                    for c in range(KC):
                        for h in range(nh):
                            nc.tensor.matmul(
                                pss[h][:, 0:hs],
                                fsb[:, c, b * 128:(b + 1) * 128],
                                wsb[:, c, h * hs:(h + 1) * hs],
                                start=(c == 0),
                                stop=(c == KC - 1),
                            )
                    for h in range(nh):
                        nc.vector.tensor_copy(
                            osb[b][:, j0 + h * hs: j0 + (h + 1) * hs],
                            pss[h][:, 0:hs],
                        )
                # output DMAs ride the ACT HWDGE ring so they don't
                # queue behind the next chunk's input DMA on SP
                if g == N_CHUNKS - 1:
                    for b in range(B // 128):
                        nc.scalar.dma_start(
                            out_d[b * 128:(b + 1) * 128, n0:n0 + nt],
                            osb[b][:, j0:j0 + nt]
                        )
                elif j0 + nt == NT:
                    for b in range(B // 128):
                        nc.scalar.dma_start(
                            out_d[b * 128:(b + 1) * 128, g * NT:(g + 1) * NT],
                            osb[b][:]
                        )
    nc.compile()
    return nc


def _run_spmd(nc, in_maps):
    last_exc = None
    for _ in range(3):  # device occasionally needs one recovery execute
        try:
            return run_bass_kernel_spmd(nc, in_maps, core_ids=list(range(N_CORES)))
        except Exception as e:  # noqa: BLE001
            last_exc = e
    raise last_exc


def _shards_ok(res, f_hat, weight, w_inv, inv_scale):
    """Guard against flaky device executes (observed: a run can silently
    return decorrelated garbage). Check a 128-column probe block per core
    against the host; caller reruns on failure."""
    ok = True
    for i in range(N_CORES):
        n0 = i * N_SHARD
        w_hat_blk = weight[n0:n0 + 128] * (w_inv[n0:n0 + 128] / W_SCALE)
        ref = f_hat @ w_hat_blk.T                   # [B, 128] fp32
        got = res.results[i]["out"][:, :128].astype(np.float32) * inv_scale
        err = np.abs(got - ref).max()
        if not np.isfinite(err) or err > 0.05:
            print(f"kernel self-check: core {i} probe absmax {err:.3e} "
                  f"-> rerun", flush=True)
            ok = False
    return ok


def _swizzle_shard(wq):
    """[N_SHARD, K] row-major -> [128, N_CHUNKS, KC, NT] so each chunk's
    per-partition line (KC*NT bytes) is contiguous."""
    # buf[p, g, c, t] = wq[g*NT + t, c*128 + p]
    v = wq.reshape(N_CHUNKS, NT, KC, 128)
    return np.ascontiguousarray(v.transpose(3, 0, 2, 1))


def kernel(features, weight, threshold):
    features = np.asarray(features, dtype=np.float32)
    weight = np.asarray(weight, dtype=np.float32)

    f_norm = np.linalg.norm(features, axis=1, keepdims=True)
    f_hat = features / np.maximum(f_norm, EPS)
    if MODE == "e3s":
        fT = np.ascontiguousarray(f_hat.T * W_SCALE).astype(ml_dtypes.float8_e3m4)
    else:
        fT = np.ascontiguousarray(f_hat.T).astype(np.float16)  # [768, 256]

    w_norm = np.linalg.norm(weight, axis=1, keepdims=True)
    w_inv = (W_SCALE / np.maximum(w_norm, EPS)).astype(np.float32)
    wnp = np.float16 if MODE == "fp16x" else ml_dtypes.float8_e3m4

    shards = []
    for i in range(N_CORES):
        n0 = i * N_SHARD
        n1 = min(n0 + N_SHARD, N_FULL)
        s = np.zeros((N_SHARD, K), dtype=wnp)
        s[: n1 - n0] = (weight[n0:n1] * w_inv[n0:n1]).astype(wnp)
        shards.append(_swizzle_shard(s))

    key = ("nc", MODE)
    if key not in _CACHED:
        _CACHED[key] = _build_bass(MODE)
    nc = _CACHED[key]

    inv_scale = np.float32(1.0 / W_SCALE)
    if MODE == "e3s":
        inv_scale = np.float32(1.0 / (W_SCALE * W_SCALE))

    in_maps = [{"fT": fT, "wT": shards[i]} for i in range(N_CORES)]
    res = _run_spmd(nc, in_maps)
    for _ in range(3):
        if _shards_ok(res, f_hat, weight, w_inv, inv_scale):
            break
        res = _run_spmd(nc, in_maps)
    _CACHED["last_result"] = res
    out = np.empty((B, N_FULL + 1), dtype=np.float32)
    for i in range(N_CORES):
        n0 = i * N_SHARD
        n1 = min(n0 + N_SHARD, N_FULL)
        out[:, n0:n1] = res.results[i]["out"][:, : n1 - n0].astype(np.float32)
        out[:, n0:n1] *= inv_scale
    out[:, N_FULL] = np.float32(threshold)
    return out
